# revision 1
# baseline (speedup 1.0000x reference)
"""CAM (channel attention module) kernel for Trainium2, 8-core SPMD.

Problem: x (16, 512, 64, 64) f32, gamma (1,) f32.
  v = x.reshape(B, C, N);  E = v @ v.T  (B x 512 x 512)
  att = softmax(rowmax(E) - E)  ==  exp(rowmin(E) - E) / rowsum(...)
  out = gamma * (att @ v) + x

Sharding: data-parallel over batch, 2 batches per core, no collectives.

Per-core per-batch pipeline:
  T: DMA v (f32) -> 128 PE transposes (f32, exact) -> DVE copies to vT (f32r)
  E: energy = vT.T @ vT in fp32r (1 cy/row at N=512), PSUM f32 accumulation
  S: softmax fused: DVE rowmin, ACT exp(min-e) with accumulated rowsum,
     DVE reciprocal, gamma folded into the per-row scale (gamma=0 -> att==0
     -> output is bit-exact x)
  A: 16 PE transposes att -> attT (f32r)
  O: out = attT.T @ v_chunks(f32r) accumulated over d; epilogue adds the
     resident exact-f32 x tile; 256KB stores per (ct, n).
"""
import sys

import numpy as np

if "/opt/trn_rl_repo" not in sys.path:
    sys.path.insert(0, "/opt/trn_rl_repo")

import concourse.bass as bass
import concourse.tile as tile
from concourse import bacc, mybir
from concourse.bass_utils import run_bass_kernel_spmd
from concourse.masks import make_identity

N_CORES = 8
B_FULL = 16
B_PER_CORE = B_FULL // N_CORES  # 2
C = 512            # channels
HW = 4096          # H*W
CT = C // 128      # 4 channel tiles
KCH = HW // 128    # 32 contraction chunks for energy
NCH = HW // 512    # 8 output column chunks

f32 = mybir.dt.float32
f32r = mybir.dt.float32r

_CACHE = {}


def _build_nc():
    nc = bacc.Bacc(None, target_bir_lowering=False)
    x_d = nc.dram_tensor("x", [B_PER_CORE, C, HW], f32, kind="ExternalInput")
    g_d = nc.dram_tensor("gamma", [1], f32, kind="ExternalInput")
    y_d = nc.dram_tensor("y", [B_PER_CORE, C, HW], f32, kind="ExternalOutput")

    with tile.TileContext(nc) as tc:
        with (
            tc.tile_pool(name="pv", bufs=1) as pv,          # v f32, 4x16KB
            tc.tile_pool(name="pvt", bufs=1) as pvt,        # vT f32r, 64KB
            tc.tile_pool(name="patt", bufs=1) as patt,      # att f32 + attT f32r
            tc.tile_pool(name="pchunk", bufs=2) as pchunk,  # f32r rhs chunks
            tc.tile_pool(name="pstage", bufs=4) as pstage,  # out staging
            tc.tile_pool(name="psmall", bufs=8) as psmall,  # per-ct scalars
            tc.tile_pool(name="psing", bufs=1) as psing,    # ident, gamma
            tc.tile_pool(name="ptp", bufs=2, space="PSUM") as ptp,    # transposes
            tc.tile_pool(name="pep", bufs=2, space="PSUM") as pep,    # energy
            tc.tile_pool(name="pop", bufs=3, space="PSUM") as pop,    # out mms
        ):
            ident = psing.tile([128, 128], f32)
            make_identity(nc, ident)
            gam = psing.tile([128, 1], f32)
            g_ap = g_d[:]
            nc.gpsimd.dma_start(
                out=gam,
                in_=bass.AP(tensor=g_ap.tensor, offset=g_ap.offset,
                            ap=[[0, 128], [1, 1]]),
            )

            for b in range(B_PER_CORE):
                # ---- T: load v, build vT via PE transposes ----
                v = []
                for ct in range(CT):
                    vt_ = pv.tile([128, HW], f32, tag=f"v{ct}")
                    nc.sync.dma_start(out=vt_, in_=x_d[b, ct * 128:(ct + 1) * 128, :])
                    v.append(vt_)
                vT = pvt.tile([128, KCH, C], f32r, tag="vT")
                for k in range(KCH):
                    tp = ptp.tile([128, C], f32, tag="tp")
                    for ct in range(CT):
                        nc.tensor.transpose(
                            tp[:, ct * 128:(ct + 1) * 128],
                            v[ct][:, k * 128:(k + 1) * 128],
                            ident,
                        )
                    nc.vector.tensor_copy(out=vT[:, k, :], in_=tp)

                # ---- E + S: energy (fp32r) + fused softmax per c-tile ----
                att = patt.tile([128, CT, C], f32, tag="att")
                for ct in range(CT):
                    ep = pep.tile([128, C], f32, tag="ep")
                    for k in range(KCH):
                        nc.tensor.matmul(
                            ep,
                            lhsT=vT[:, k, ct * 128:(ct + 1) * 128],
                            rhs=vT[:, k, :],
                            start=(k == 0),
                            stop=(k == KCH - 1),
                        )
                    mn = psmall.tile([128, 1], f32, tag="mn")
                    nc.vector.tensor_reduce(
                        out=mn, in_=ep, axis=mybir.AxisListType.X,
                        op=mybir.AluOpType.min,
                    )
                    ss = psmall.tile([128, 1], f32, tag="ss")
                    nc.scalar.activation(
                        out=att[:, ct, :], in_=ep,
                        func=mybir.ActivationFunctionType.Exp,
                        bias=mn, scale=-1.0, accum_out=ss,
                    )
                    rg = psmall.tile([128, 1], f32, tag="rg")
                    nc.vector.reciprocal(out=rg, in_=ss)
                    nc.vector.tensor_mul(out=rg, in0=rg, in1=gam)
                    nc.vector.tensor_scalar_mul(att[:, ct, :], att[:, ct, :], rg)

                # ---- A: transpose att -> attT (f32r) ----
                attT = patt.tile([128, CT, C], f32r, tag="attT")
                for dt in range(CT):
                    tp = ptp.tile([128, C], f32, tag="tp")
                    for ct in range(CT):
                        nc.tensor.transpose(
                            tp[:, ct * 128:(ct + 1) * 128],
                            att[:, ct, dt * 128:(dt + 1) * 128],
                            ident,
                        )
                    nc.vector.tensor_copy(out=attT[:, dt, :], in_=tp)

                # ---- O: out = attT.T @ v_f32r + x, streamed per n-chunk ----
                for n in range(NCH):
                    nsl = slice(n * 512, (n + 1) * 512)
                    chunks = []
                    for dt in range(CT):
                        ch = pchunk.tile([128, 512], f32r, tag=f"ch{dt}")
                        nc.vector.tensor_copy(out=ch, in_=v[dt][:, nsl])
                        chunks.append(ch)
                    for ct in range(CT):
                        op = pop.tile([128, 512], f32, tag="op")
                        for dt in range(CT):
                            nc.tensor.matmul(
                                op,
                                lhsT=attT[:, dt, ct * 128:(ct + 1) * 128],
                                rhs=chunks[dt],
                                start=(dt == 0),
                                stop=(dt == CT - 1),
                            )
                        st = pstage.tile([128, 512], f32, tag="st")
                        nc.vector.tensor_add(out=st, in0=op, in1=v[ct][:, nsl])
                        nc.sync.dma_start(
                            out=y_d[b, ct * 128:(ct + 1) * 128, nsl], in_=st,
                        )

    nc.compile()
    return nc


def kernel(x: np.ndarray, gamma: np.ndarray) -> np.ndarray:
    x = np.ascontiguousarray(np.asarray(x, dtype=np.float32))
    gamma = np.ascontiguousarray(np.asarray(gamma, dtype=np.float32))
    B, Cc, H, W = x.shape
    xv = x.reshape(B, Cc, H * W)

    if "nc" not in _CACHE:
        _CACHE["nc"] = _build_nc()
    nc = _CACHE["nc"]

    in_maps = [
        {"x": xv[i * B_PER_CORE:(i + 1) * B_PER_CORE], "gamma": gamma}
        for i in range(N_CORES)
    ]
    res = run_bass_kernel_spmd(nc, in_maps, list(range(N_CORES)))
    y = np.concatenate([res.results[i]["y"] for i in range(N_CORES)], axis=0)
    return y.reshape(B, Cc, H, W).astype(np.float32)


# revision 34
# speedup vs baseline: 40028.6135x; 40028.6135x over previous
"""CAM (channel attention module) kernel for Trainium2, 8-core SPMD.

Problem: x (16, 512, 64, 64) f32, gamma (1,) f32.
  v = x.reshape(B, C, N);  E = v @ v.T  (B x 512 x 512)
  att = softmax(rowmax(E) - E)  ==  exp(rowmin(E) - E) / rowsum(...)
  out = gamma * (att @ v) + x

Sharding: data-parallel over batch, 2 batches per core, no collectives.

Per-core per-batch pipeline (matmul operands in fp16 = TF32-class accuracy,
all accumulation and the x-residual in exact f32). Tile-dependency tracking
is whole-tile, so tensors are split into tiles matching their consumers'
granularity (v in halves, att per row-tile, attT per d-tile):
  T: DMA v halves (f32, spread over sync/gpsimd/scalar DMA queues,
     v0/v1/v2 double-buffered cross-batch) -> gpsimd/DVE f32->fp16 copies
     per half -> 128 PE transposes (fp16, 1 cy/row) -> DVE copies to vT.
  E: energy = vT.T @ vT, fp16 operands, f32 PSUM accumulation. Symmetry:
     row-tile ct computes only columns >= ct*128 (N=512/384/256/128, the
     upper triangle); the 6 missing blocks are mirrored from earlier rows
     by PE transpose straight back into the PSUM row.
  S: fused softmax of (rowmin(E) - E) [stable form of softmax(rowmax-E)]:
     DVE rowmin, ACT exp with accumulated rowsum, DVE reciprocal; gamma is
     folded into the per-row scale, so gamma==0 gives att==0 and the output
     is bit-exact x.
  A: 16 PE transposes att -> attT (fp16), per-dt tiles.
  O: out = attT.T @ chunks(fp16 of v) accumulated over d-chunks into f32
     PSUM; DVE epilogue adds the exact-f32 x tile; 256KB stores. The last
     n-iter reads x for ct3 from a re-streamed tile so v3's second half
     releases early for the next batch's load.
"""
import sys

import numpy as np

if "/opt/trn_rl_repo" not in sys.path:
    sys.path.insert(0, "/opt/trn_rl_repo")

import concourse.bass as bass
import concourse.tile as tile
from concourse import bacc, mybir
from concourse.bass_utils import run_bass_kernel_spmd
from concourse.masks import make_identity

N_CORES = 8
B_FULL = 16
B_PER_CORE = B_FULL // N_CORES  # 2
C = 512            # channels
HW = 4096          # H*W
CT = C // 128      # 4 channel tiles
KCH = HW // 128    # 32 contraction chunks for energy
NCH = HW // 512    # 8 output column chunks
HHW = HW // 2      # half of H*W (v half-tile width)

f32 = mybir.dt.float32
f16 = mybir.dt.float16

_CACHE = {}

# engine that converts f32->fp16 for each ct's halves in the T phase
CONV_ENGINES = {0: "vector", 1: "gpsimd", 2: "gpsimd", 3: "gpsimd"}
# DMA queue that loads each ct's halves
LOAD_ENGINES = {0: "sync", 1: "gpsimd", 2: "scalar", 3: "scalar"}


def _build_nc(reps: int = 1):
    nc = bacc.Bacc(None, target_bir_lowering=False)
    x_d = nc.dram_tensor("x", [B_PER_CORE, C, HW], f32, kind="ExternalInput")
    g_d = nc.dram_tensor("gamma", [1], f32, kind="ExternalInput")
    y_d = nc.dram_tensor("y", [B_PER_CORE, C, HW], f32, kind="ExternalOutput")

    with tile.TileContext(nc) as tc:
        with (
            tc.tile_pool(name="pvA", bufs=2) as pvA,        # v0/v1/v2 halves
            tc.tile_pool(name="pvB", bufs=1) as pvB,        # v3 halves
            tc.tile_pool(name="pvt", bufs=1) as pvt,        # vT fp16 32KB
            tc.tile_pool(name="pv16", bufs=2) as pv16,      # fp16 v half-tiles
            tc.tile_pool(name="patt", bufs=1) as patt,      # att/attT fp16
            tc.tile_pool(name="pchunk", bufs=2) as pchunk,  # fp16 rhs chunks
            tc.tile_pool(name="pstage", bufs=4) as pstage,  # out staging
            tc.tile_pool(name="pstream", bufs=1) as pstream,  # x re-stream
            tc.tile_pool(name="psmall", bufs=8) as psmall,  # per-ct scalars
            tc.tile_pool(name="pmir", bufs=1) as pmir,      # mirror blocks
            tc.tile_pool(name="psing", bufs=1) as psing,    # ident, gamma
            tc.tile_pool(name="ptp", bufs=2, space="PSUM") as ptp,
            tc.tile_pool(name="pep", bufs=2, space="PSUM") as pep,
            tc.tile_pool(name="pop", bufs=4, space="PSUM") as pop,
        ):
            ident = psing.tile([128, 128], f32)
            make_identity(nc, ident)
            ident16 = psing.tile([128, 128], f16)
            nc.vector.tensor_copy(out=ident16, in_=ident)
            gam = psing.tile([128, 1], f32)
            g_ap = g_d[:]
            nc.gpsimd.dma_start(
                out=gam,
                in_=bass.AP(tensor=g_ap.tensor, offset=g_ap.offset,
                            ap=[[0, 128], [1, 1]]),
            )

            def load_vh(b, ct, h, parts=1):
                pool = pvB if ct == 3 else pvA
                t_ = pool.tile([128, HHW], f32, tag=f"v{ct}h{h}")
                eng = getattr(nc, LOAD_ENGINES[ct])
                w = HHW // parts
                for p in range(parts):
                    eng.dma_start(
                        out=t_[:, p * w:(p + 1) * w],
                        in_=x_d[b, ct * 128:(ct + 1) * 128,
                                h * HHW + p * w:h * HHW + (p + 1) * w],
                    )
                return t_

            def load_vct(b, ct, parts=1):
                return [load_vh(b, ct, 0, parts), load_vh(b, ct, 1, parts)]

            batches = [bb for _ in range(reps) for bb in range(B_PER_CORE)]
            v012 = [load_vct(batches[0], 0, parts=2),
                    load_vct(batches[0], 1),
                    load_vct(batches[0], 2)]

            for bi, b in enumerate(batches):
                v = v012 + [load_vct(b, 3)]

                def xcol(dt, n):
                    """f32 x slice [128, 512] for (row-tile dt, n-chunk n)."""
                    h, lo = divmod(n * 512, HHW)
                    return v[dt][h][:, lo:lo + 512]

                # ---- T: fp16 convert + PE transposes (1 cy/row) ----
                vT = pvt.tile([128, KCH, C], f16, tag="vT")
                for ct in range(CT):
                    eng = getattr(nc, CONV_ENGINES[ct])
                    for h in range(2):
                        v16 = pv16.tile([128, HHW], f16, tag=f"v16_{h}")
                        eng.tensor_copy(out=v16, in_=v[ct][h])
                        for jb in range(4):
                            tp = ptp.tile([128, 4, 128], f16, tag="tp")
                            for ks in range(4):
                                k = jb * 4 + ks
                                nc.tensor.transpose(
                                    tp[:, ks, :],
                                    v16[:, k * 128:(k + 1) * 128],
                                    ident16,
                                )
                            kk = h * 16 + jb * 4
                            nc.vector.tensor_copy(
                                out=vT[:, kk:kk + 4, ct * 128:(ct + 1) * 128],
                                in_=tp,
                            )
                # prefetch next batch's double-buffered v0/v1/v2
                if bi + 1 < len(batches):
                    v012 = [load_vct(batches[bi + 1], 0),
                            load_vct(batches[bi + 1], 1),
                            load_vct(batches[bi + 1], 2)]

                # ---- E + S: energy (fp16 operands, f32 accum) + softmax ----
                att = [None] * CT
                mirror_src = {}  # (ct, dt) -> sbuf [128,128] copy of E block
                for ct in range(CT):
                    off = ct * 128
                    ep = pep.tile([128, C], f32, tag="ep")
                    for k in range(KCH):
                        nc.tensor.matmul(
                            ep[:, off:],
                            lhsT=vT[:, k, ct * 128:(ct + 1) * 128],
                            rhs=vT[:, k, off:],
                            start=(k == 0),
                            stop=(k == KCH - 1),
                        )
                    # stash blocks that later rows mirror
                    for (dst, src) in (((1, 0), (0, 1)), ((2, 0), (0, 2)),
                                       ((2, 1), (1, 2)), ((3, 0), (0, 3)),
                                       ((3, 1), (1, 3)), ((3, 2), (2, 3))):
                        if src[0] == ct:
                            sb = pmir.tile([128, 128], f32,
                                           tag=f"mir{dst[0]}{dst[1]}")
                            nc.vector.tensor_copy(
                                out=sb,
                                in_=ep[:, src[1] * 128:(src[1] + 1) * 128],
                            )
                            mirror_src[dst] = sb
                    for dt in range(ct):
                        nc.tensor.transpose(
                            ep[:, dt * 128:(dt + 1) * 128],
                            mirror_src[(ct, dt)], ident,
                        )
                    mn = psmall.tile([128, 1], f32, tag="mn")
                    nc.vector.tensor_reduce(
                        out=mn, in_=ep, axis=mybir.AxisListType.X,
                        op=mybir.AluOpType.min,
                    )
                    a_ = patt.tile([128, C], f16, tag=f"att{ct}")
                    ss = psmall.tile([128, 1], f32, tag="ss")
                    nc.scalar.activation(
                        out=a_, in_=ep,
                        func=mybir.ActivationFunctionType.Exp,
                        bias=mn, scale=-1.0, accum_out=ss,
                    )
                    rg = psmall.tile([128, 1], f32, tag="rg")
                    nc.vector.reciprocal(out=rg, in_=ss)
                    nc.vector.tensor_mul(out=rg, in0=rg, in1=gam)
                    nc.vector.tensor_scalar_mul(a_, a_, rg)
                    att[ct] = a_

                # ---- A: transpose att -> attT (fp16, per-dt tiles) ----
                attT = []
                for dt in range(CT):
                    tp = ptp.tile([128, C], f16, tag="tp")
                    for ct in range(CT):
                        nc.tensor.transpose(
                            tp[:, ct * 128:(ct + 1) * 128],
                            att[ct][:, dt * 128:(dt + 1) * 128],
                            ident16,
                        )
                    aT = patt.tile([128, C], f16, tag=f"attT{dt}")
                    nc.vector.tensor_copy(out=aT, in_=tp)
                    attT.append(aT)

                # ---- O: out = attT.T @ chunks + x, per 512-wide n-chunk ----
                # last n-iter x for ct3 comes from a re-streamed tile so
                # v3's second half releases one iteration early
                xs3 = pstream.tile([128, 512], f32, tag="xs3")
                nc.sync.dma_start(
                    out=xs3, in_=x_d[b, 384:512, (NCH - 1) * 512:],
                )

                def xsrc(dt, n):
                    if n == NCH - 1 and dt == 3:
                        return xs3
                    return xcol(dt, n)

                for n in range(NCH):
                    nsl = slice(n * 512, (n + 1) * 512)
                    chunks = []
                    for dt in range(CT):
                        ch = pchunk.tile([128, 512], f16, tag=f"ch{dt}")
                        nc.gpsimd.tensor_copy(out=ch, in_=xsrc(dt, n))
                        chunks.append(ch)
                    for ct in range(CT):
                        op = pop.tile([128, 512], f32, tag="op")
                        for dt in range(CT):
                            nc.tensor.matmul(
                                op,
                                lhsT=attT[dt][:, ct * 128:(ct + 1) * 128],
                                rhs=chunks[dt],
                                start=(dt == 0),
                                stop=(dt == CT - 1),
                            )
                        st = pstage.tile([128, 512], f32, tag="st")
                        nc.vector.tensor_add(out=st, in0=op, in1=xsrc(ct, n))
                        nc.sync.dma_start(
                            out=y_d[b, ct * 128:(ct + 1) * 128, nsl], in_=st,
                        )

    nc.compile()
    return nc


def kernel(x: np.ndarray, gamma: np.ndarray) -> np.ndarray:
    x = np.ascontiguousarray(np.asarray(x, dtype=np.float32))
    gamma = np.ascontiguousarray(np.asarray(gamma, dtype=np.float32))
    B, Cc, H, W = x.shape
    xv = x.reshape(B, Cc, H * W)

    if "nc" not in _CACHE:
        _CACHE["nc"] = _build_nc()
    nc = _CACHE["nc"]

    in_maps = [
        {"x": xv[i * B_PER_CORE:(i + 1) * B_PER_CORE], "gamma": gamma}
        for i in range(N_CORES)
    ]
    res = run_bass_kernel_spmd(nc, in_maps, list(range(N_CORES)))
    y = np.concatenate([res.results[i]["y"] for i in range(N_CORES)], axis=0)
    return y.reshape(B, Cc, H, W).astype(np.float32)


# revision 41
# speedup vs baseline: 41465.4173x; 1.0359x over previous
"""CAM (channel attention module) kernel for Trainium2, 8-core SPMD.

Problem: x (16, 512, 64, 64) f32, gamma (1,) f32.
  v = x.reshape(B, C, N);  E = v @ v.T  (B x 512 x 512)
  att = softmax(rowmax(E) - E)  ==  exp(rowmin(E) - E) / rowsum(...)
  out = gamma * (att @ v) + x

Sharding: data-parallel over batch, 2 batches per core, no collectives.

Per-core per-batch pipeline (matmul operands in fp16 = TF32-class accuracy,
all accumulation and the x-residual in exact f32). Tile-dependency tracking
is whole-tile, so tensors are split into tiles matching their consumers'
granularity (v in halves, att per row-tile, attT per d-tile):
  T: DMA v halves (f32, spread over sync/gpsimd/scalar DMA queues,
     v0/v1/v2 double-buffered cross-batch) -> gpsimd/DVE f32->fp16 copies
     per half -> 128 PE transposes (fp16, 1 cy/row) -> DVE copies to vT.
  E: energy = vT.T @ vT, fp16 operands, f32 PSUM accumulation. Symmetry:
     row-tile ct computes only columns >= ct*128 (N=512/384/256/128, the
     upper triangle); the 6 missing blocks are mirrored from earlier rows
     by PE transpose straight back into the PSUM row.
  S: fused softmax of (rowmin(E) - E) [stable form of softmax(rowmax-E)]:
     DVE rowmin, ACT exp with accumulated rowsum, DVE reciprocal; gamma is
     folded into the per-row scale, so gamma==0 gives att==0 and the output
     is bit-exact x.
  A: 16 PE transposes att -> attT (fp16), per-dt tiles.
  O: out = attT.T @ chunks(fp16 of v) accumulated over d-chunks into f32
     PSUM; DVE epilogue adds the exact-f32 x tile; 256KB stores. The last
     n-iter reads x for ct3 from a re-streamed tile so v3's second half
     releases early for the next batch's load.
"""
import sys

import numpy as np

if "/opt/trn_rl_repo" not in sys.path:
    sys.path.insert(0, "/opt/trn_rl_repo")

import concourse.bass as bass
import concourse.tile as tile
from concourse import bacc, mybir
from concourse.bass_utils import run_bass_kernel_spmd
from concourse.masks import make_identity

N_CORES = 8
B_FULL = 16
B_PER_CORE = B_FULL // N_CORES  # 2
C = 512            # channels
HW = 4096          # H*W
CT = C // 128      # 4 channel tiles
KCH = HW // 128    # 32 contraction chunks for energy
NCH = HW // 512    # 8 output column chunks
HHW = HW // 2      # half of H*W (v half-tile width)

f32 = mybir.dt.float32
f16 = mybir.dt.float16

_CACHE = {}

# engine that converts f32->fp16 for each ct's halves in the T phase
CONV_ENGINES = {0: "vector", 1: "gpsimd", 2: "gpsimd", 3: "gpsimd"}
# DMA queue that loads each ct's halves
LOAD_ENGINES = {0: "sync", 1: "gpsimd", 2: "scalar", 3: "scalar"}


def _build_nc(reps: int = 1):
    nc = bacc.Bacc(None, target_bir_lowering=False)
    x_d = nc.dram_tensor("x", [B_PER_CORE, C, HW], f32, kind="ExternalInput")
    g_d = nc.dram_tensor("gamma", [1], f32, kind="ExternalInput")
    y_d = nc.dram_tensor("y", [B_PER_CORE, C, HW], f32, kind="ExternalOutput")

    with tile.TileContext(nc) as tc:
        with (
            tc.tile_pool(name="pvA", bufs=2) as pvA,        # v0/v1/v2 halves
            tc.tile_pool(name="pvB", bufs=1) as pvB,        # v3 halves
            tc.tile_pool(name="pvt", bufs=1) as pvt,        # vT fp16 32KB
            tc.tile_pool(name="pv16", bufs=2) as pv16,      # fp16 v half-tiles
            tc.tile_pool(name="patt", bufs=1) as patt,      # att/attT fp16
            tc.tile_pool(name="pchunk", bufs=2) as pchunk,  # fp16 rhs chunks
            tc.tile_pool(name="pstage", bufs=4) as pstage,  # out staging
            tc.tile_pool(name="pstream", bufs=1) as pstream,  # x re-stream
            tc.tile_pool(name="psmall", bufs=8) as psmall,  # per-ct scalars
            tc.tile_pool(name="pmir", bufs=1) as pmir,      # mirror blocks
            tc.tile_pool(name="psing", bufs=1) as psing,    # ident, gamma
            tc.tile_pool(name="ptp", bufs=2, space="PSUM") as ptp,
            tc.tile_pool(name="pep", bufs=2, space="PSUM") as pep,
            tc.tile_pool(name="pop", bufs=4, space="PSUM") as pop,
        ):
            ident = psing.tile([128, 128], f32)
            make_identity(nc, ident)
            ident16 = psing.tile([128, 128], f16)
            nc.vector.tensor_copy(out=ident16, in_=ident)
            gam = psing.tile([128, 1], f32)
            g_ap = g_d[:]
            nc.gpsimd.dma_start(
                out=gam,
                in_=bass.AP(tensor=g_ap.tensor, offset=g_ap.offset,
                            ap=[[0, 128], [1, 1]]),
            )

            def load_vh(b, ct, h):
                pool = pvB if ct == 3 else pvA
                t_ = pool.tile([128, HHW], f32, tag=f"v{ct}h{h}")
                getattr(nc, LOAD_ENGINES[ct]).dma_start(
                    out=t_,
                    in_=x_d[b, ct * 128:(ct + 1) * 128, h * HHW:(h + 1) * HHW],
                )
                return t_

            def load_group(b):
                tiles = [[None, None] for _ in range(3)]
                for h in (0, 1):
                    for ct in (0, 1, 2):
                        tiles[ct][h] = load_vh(b, ct, h)
                return tiles

            batches = [bb for _ in range(reps) for bb in range(B_PER_CORE)]
            v012 = load_group(batches[0])


            for bi, b in enumerate(batches):
                v = list(v012) + [[load_vh(b, 3, 0), load_vh(b, 3, 1)]]

                def xcol(dt, n):
                    """f32 x slice [128, 512] for (row-tile dt, n-chunk n)."""
                    h, lo = divmod(n * 512, HHW)
                    return v[dt][h][:, lo:lo + 512]

                # ---- T: fp16 convert + PE transposes (1 cy/row).
                # vT is split into 4 k-quarter tiles so the energy chains
                # can start on early quarters while the last quarters'
                # PSUM->SBUF copies are still draining on DVE.
                vTq = [pvt.tile([128, 8, C], f16, tag=f"vTq{q}",
                                name=f"vTq{q}") for q in range(4)]

                def vT(k):
                    return vTq[k // 8][:, k % 8, :]

                for ct in range(CT):
                    eng = getattr(nc, CONV_ENGINES[ct])
                    for h in range(2):
                        v16 = pv16.tile([128, HHW], f16, tag=f"v16_{h}")
                        eng.tensor_copy(out=v16, in_=v[ct][h])
                        for q2 in range(2):
                            tp = ptp.tile([128, 8, 128], f16, tag="tp")
                            for ks in range(8):
                                k = h * 16 + q2 * 8 + ks
                                kl = q2 * 8 + ks
                                nc.tensor.transpose(
                                    tp[:, ks, :],
                                    v16[:, kl * 128:(kl + 1) * 128],
                                    ident16,
                                )
                            nc.vector.tensor_copy(
                                out=vTq[h * 2 + q2][:, :,
                                                    ct * 128:(ct + 1) * 128],
                                in_=tp,
                            )
                # prefetch next batch's double-buffered v0/v1/v2
                if bi + 1 < len(batches):
                    v012 = load_group(batches[bi + 1])

                # ---- E + S: energy (fp16 operands, f32 accum) + softmax ----
                att = [None] * CT
                mirror_src = {}  # (ct, dt) -> sbuf [128,128] copy of E block
                for ct in range(CT):
                    off = ct * 128
                    ep = pep.tile([128, C], f32, tag="ep")
                    for k in range(KCH):
                        nc.tensor.matmul(
                            ep[:, off:],
                            lhsT=vT(k)[:, ct * 128:(ct + 1) * 128],
                            rhs=vT(k)[:, off:],
                            start=(k == 0),
                            stop=(k == KCH - 1),
                        )
                    # stash blocks that later rows mirror
                    for (dst, src) in (((1, 0), (0, 1)), ((2, 0), (0, 2)),
                                       ((2, 1), (1, 2)), ((3, 0), (0, 3)),
                                       ((3, 1), (1, 3)), ((3, 2), (2, 3))):
                        if src[0] == ct:
                            sb = pmir.tile([128, 128], f32,
                                           tag=f"mir{dst[0]}{dst[1]}")
                            nc.vector.tensor_copy(
                                out=sb,
                                in_=ep[:, src[1] * 128:(src[1] + 1) * 128],
                            )
                            mirror_src[dst] = sb
                    for dt in range(ct):
                        nc.tensor.transpose(
                            ep[:, dt * 128:(dt + 1) * 128],
                            mirror_src[(ct, dt)], ident,
                        )
                    mn = psmall.tile([128, 1], f32, tag="mn")
                    nc.vector.tensor_reduce(
                        out=mn, in_=ep, axis=mybir.AxisListType.X,
                        op=mybir.AluOpType.min,
                    )
                    a_ = patt.tile([128, C], f16, tag=f"att{ct}")
                    ss = psmall.tile([128, 1], f32, tag="ss")
                    nc.scalar.activation(
                        out=a_, in_=ep,
                        func=mybir.ActivationFunctionType.Exp,
                        bias=mn, scale=-1.0, accum_out=ss,
                    )
                    rg = psmall.tile([128, 1], f32, tag="rg")
                    nc.vector.reciprocal(out=rg, in_=ss)
                    nc.vector.tensor_mul(out=rg, in0=rg, in1=gam)
                    nc.vector.tensor_scalar_mul(a_, a_, rg)
                    att[ct] = a_

                # ---- A: transpose att -> attT (fp16, per-dt tiles) ----
                attT = []
                for dt in range(CT):
                    tp = ptp.tile([128, C], f16, tag="tp")
                    for ct in range(CT):
                        nc.tensor.transpose(
                            tp[:, ct * 128:(ct + 1) * 128],
                            att[ct][:, dt * 128:(dt + 1) * 128],
                            ident16,
                        )
                    aT = patt.tile([128, C], f16, tag=f"attT{dt}")
                    nc.vector.tensor_copy(out=aT, in_=tp)
                    attT.append(aT)

                # ---- O: out = attT.T @ chunks + x, per 512-wide n-chunk ----
                # last n-iter x for ct3 comes from a re-streamed tile so
                # v3's second half releases one iteration early
                xs3 = pstream.tile([128, 512], f32, tag="xs3")
                nc.sync.dma_start(
                    out=xs3, in_=x_d[b, 384:512, (NCH - 1) * 512:],
                )

                def xsrc(dt, n):
                    if n == NCH - 1 and dt == 3:
                        return xs3
                    return xcol(dt, n)

                for n in range(NCH):
                    nsl = slice(n * 512, (n + 1) * 512)
                    chunks = []
                    for dt in range(CT):
                        ch = pchunk.tile([128, 512], f16, tag=f"ch{dt}")
                        nc.gpsimd.tensor_copy(out=ch, in_=xsrc(dt, n))
                        chunks.append(ch)
                    for ct in range(CT):
                        op = pop.tile([128, 512], f32, tag="op")
                        for dt in range(CT):
                            nc.tensor.matmul(
                                op,
                                lhsT=attT[dt][:, ct * 128:(ct + 1) * 128],
                                rhs=chunks[dt],
                                start=(dt == 0),
                                stop=(dt == CT - 1),
                            )
                        st = pstage.tile([128, 512], f32, tag="st")
                        nc.vector.tensor_add(out=st, in0=op, in1=xsrc(ct, n))
                        nc.sync.dma_start(
                            out=y_d[b, ct * 128:(ct + 1) * 128, nsl], in_=st,
                        )

    nc.compile()
    return nc


def kernel(x: np.ndarray, gamma: np.ndarray) -> np.ndarray:
    x = np.ascontiguousarray(np.asarray(x, dtype=np.float32))
    gamma = np.ascontiguousarray(np.asarray(gamma, dtype=np.float32))
    B, Cc, H, W = x.shape
    xv = x.reshape(B, Cc, H * W)

    if "nc" not in _CACHE:
        _CACHE["nc"] = _build_nc()
    nc = _CACHE["nc"]

    in_maps = [
        {"x": xv[i * B_PER_CORE:(i + 1) * B_PER_CORE], "gamma": gamma}
        for i in range(N_CORES)
    ]
    res = run_bass_kernel_spmd(nc, in_maps, list(range(N_CORES)))
    y = np.concatenate([res.results[i]["y"] for i in range(N_CORES)], axis=0)
    return y.reshape(B, Cc, H, W).astype(np.float32)


# revision 44
# speedup vs baseline: 41746.8062x; 1.0068x over previous
"""CAM (channel attention module) kernel for Trainium2, 8-core SPMD.

Problem: x (16, 512, 64, 64) f32, gamma (1,) f32.
  v = x.reshape(B, C, N);  E = v @ v.T  (B x 512 x 512)
  att = softmax(rowmax(E) - E)  ==  exp(rowmin(E) - E) / rowsum(...)
  out = gamma * (att @ v) + x

Sharding: data-parallel over batch, 2 batches per core, no collectives.

Per-core per-batch pipeline (matmul operands in fp16 = TF32-class accuracy,
all accumulation and the x-residual in exact f32). Tile-dependency tracking
is whole-tile, so tensors are split into tiles matching their consumers'
granularity (v in halves, att per row-tile, attT per d-tile):
  T: DMA v halves (f32, spread over sync/gpsimd/scalar DMA queues,
     v0/v1/v2 double-buffered cross-batch) -> gpsimd/DVE f32->fp16 copies
     per half -> 128 PE transposes (fp16, 1 cy/row) -> DVE copies to vT.
  E: energy = vT.T @ vT, fp16 operands, f32 PSUM accumulation. Symmetry:
     row-tile ct computes only columns >= ct*128 (N=512/384/256/128, the
     upper triangle); the 6 missing blocks are mirrored from earlier rows
     by PE transpose straight back into the PSUM row.
  S: fused softmax of (rowmin(E) - E) [stable form of softmax(rowmax-E)]:
     DVE rowmin, ACT exp with accumulated rowsum, DVE reciprocal; gamma is
     folded into the per-row scale, so gamma==0 gives att==0 and the output
     is bit-exact x.
  A: 16 PE transposes att -> attT (fp16), per-dt tiles.
  O: out = attT.T @ chunks(fp16 of v) accumulated over d-chunks into f32
     PSUM; DVE epilogue adds the exact-f32 x tile; 256KB stores. The last
     n-iter reads x for ct3 from a re-streamed tile so v3's second half
     releases early for the next batch's load.
"""
import sys

import numpy as np

if "/opt/trn_rl_repo" not in sys.path:
    sys.path.insert(0, "/opt/trn_rl_repo")

import concourse.bass as bass
import concourse.tile as tile
from concourse import bacc, mybir
from concourse.bass_utils import run_bass_kernel_spmd
from concourse.masks import make_identity

N_CORES = 8
B_FULL = 16
B_PER_CORE = B_FULL // N_CORES  # 2
C = 512            # channels
HW = 4096          # H*W
CT = C // 128      # 4 channel tiles
KCH = HW // 128    # 32 contraction chunks for energy
NCH = HW // 512    # 8 output column chunks
HHW = HW // 2      # half of H*W (v half-tile width)

f32 = mybir.dt.float32
f16 = mybir.dt.float16

_CACHE = {}

# engine that converts f32->fp16 for each ct's halves in the T phase
CONV_ENGINES = {0: "vector", 1: "gpsimd", 2: "gpsimd", 3: "gpsimd"}
# DMA queue that loads each ct's halves
LOAD_ENGINES = {0: "sync", 1: "gpsimd", 2: "scalar", 3: "scalar"}


def _build_nc(reps: int = 1):
    nc = bacc.Bacc(None, target_bir_lowering=False)
    x_d = nc.dram_tensor("x", [B_PER_CORE, C, HW], f32, kind="ExternalInput")
    g_d = nc.dram_tensor("gamma", [1], f32, kind="ExternalInput")
    y_d = nc.dram_tensor("y", [B_PER_CORE, C, HW], f32, kind="ExternalOutput")

    with tile.TileContext(nc) as tc:
        with (
            tc.tile_pool(name="pvA", bufs=2) as pvA,        # v0/v1/v2 halves
            tc.tile_pool(name="pvB", bufs=1) as pvB,        # v3 halves
            tc.tile_pool(name="pvt", bufs=1) as pvt,        # vT fp16 32KB
            tc.tile_pool(name="pv16", bufs=2) as pv16,      # fp16 v half-tiles
            tc.tile_pool(name="patt", bufs=1) as patt,      # att/attT fp16
            tc.tile_pool(name="pchunk", bufs=2) as pchunk,  # fp16 rhs chunks
            tc.tile_pool(name="pstage", bufs=4) as pstage,  # out staging
            tc.tile_pool(name="pstream", bufs=1) as pstream,  # x re-stream
            tc.tile_pool(name="psmall", bufs=8) as psmall,  # per-ct scalars
            tc.tile_pool(name="pmir", bufs=1) as pmir,      # mirror blocks
            tc.tile_pool(name="psing", bufs=1) as psing,    # ident, gamma
            tc.tile_pool(name="ptp", bufs=2, space="PSUM") as ptp,
            tc.tile_pool(name="pep", bufs=2, space="PSUM") as pep,
            tc.tile_pool(name="pop", bufs=4, space="PSUM") as pop,
        ):
            ident = psing.tile([128, 128], f32)
            make_identity(nc, ident)
            ident16 = psing.tile([128, 128], f16)
            nc.vector.tensor_copy(out=ident16, in_=ident)
            gam = psing.tile([128, 1], f32)

            def load_gamma():
                g_ap = g_d[:]
                nc.gpsimd.dma_start(
                    out=gam,
                    in_=bass.AP(tensor=g_ap.tensor, offset=g_ap.offset,
                                ap=[[0, 128], [1, 1]]),
                )

            def load_vh(b, ct, h):
                pool = pvB if ct == 3 else pvA
                t_ = pool.tile([128, HHW], f32, tag=f"v{ct}h{h}")
                getattr(nc, LOAD_ENGINES[ct]).dma_start(
                    out=t_,
                    in_=x_d[b, ct * 128:(ct + 1) * 128, h * HHW:(h + 1) * HHW],
                )
                return t_

            def load_group(b):
                tiles = [[None, None] for _ in range(3)]
                for h in (0, 1):
                    for ct in (0, 1, 2):
                        tiles[ct][h] = load_vh(b, ct, h)
                return tiles

            batches = [bb for _ in range(reps) for bb in range(B_PER_CORE)]
            v012 = load_group(batches[0])


            for bi, b in enumerate(batches):
                v = list(v012) + [[load_vh(b, 3, 0), load_vh(b, 3, 1)]]

                def xcol(dt, n):
                    """f32 x slice [128, 512] for (row-tile dt, n-chunk n)."""
                    h, lo = divmod(n * 512, HHW)
                    return v[dt][h][:, lo:lo + 512]

                # ---- T: fp16 convert + PE transposes (1 cy/row).
                # vT is split into 4 k-quarter tiles so the energy chains
                # can start on early quarters while the last quarters'
                # PSUM->SBUF copies are still draining on DVE.
                vTq = [pvt.tile([128, 8, C], f16, tag=f"vTq{q}",
                                name=f"vTq{q}") for q in range(4)]

                def vT(k):
                    return vTq[k // 8][:, k % 8, :]

                for ct in range(CT):
                    eng = getattr(nc, CONV_ENGINES[ct])
                    for h in range(2):
                        v16 = pv16.tile([128, HHW], f16, tag=f"v16_{h}")
                        eng.tensor_copy(out=v16, in_=v[ct][h])
                        for q2 in range(2):
                            tp = ptp.tile([128, 8, 128], f16, tag="tp")
                            for ks in range(8):
                                k = h * 16 + q2 * 8 + ks
                                kl = q2 * 8 + ks
                                nc.tensor.transpose(
                                    tp[:, ks, :],
                                    v16[:, kl * 128:(kl + 1) * 128],
                                    ident16,
                                )
                            nc.vector.tensor_copy(
                                out=vTq[h * 2 + q2][:, :,
                                                    ct * 128:(ct + 1) * 128],
                                in_=tp,
                            )
                if bi == 0:
                    load_gamma()  # emitted late so it never delays v loads
                # prefetch next batch's double-buffered v0/v1/v2
                if bi + 1 < len(batches):
                    v012 = load_group(batches[bi + 1])

                # ---- E + S: energy (fp16 operands, f32 accum) + softmax ----
                att = [None] * CT
                mirror_src = {}  # (ct, dt) -> sbuf [128,128] copy of E block
                for ct in range(CT):
                    off = ct * 128
                    ep = pep.tile([128, C], f32, tag="ep")
                    for k in range(KCH):
                        nc.tensor.matmul(
                            ep[:, off:],
                            lhsT=vT(k)[:, ct * 128:(ct + 1) * 128],
                            rhs=vT(k)[:, off:],
                            start=(k == 0),
                            stop=(k == KCH - 1),
                        )
                    # stash blocks that later rows mirror
                    for (dst, src) in (((1, 0), (0, 1)), ((2, 0), (0, 2)),
                                       ((2, 1), (1, 2)), ((3, 0), (0, 3)),
                                       ((3, 1), (1, 3)), ((3, 2), (2, 3))):
                        if src[0] == ct:
                            sb = pmir.tile([128, 128], f32,
                                           tag=f"mir{dst[0]}{dst[1]}")
                            nc.vector.tensor_copy(
                                out=sb,
                                in_=ep[:, src[1] * 128:(src[1] + 1) * 128],
                            )
                            mirror_src[dst] = sb
                    for dt in range(ct):
                        nc.tensor.transpose(
                            ep[:, dt * 128:(dt + 1) * 128],
                            mirror_src[(ct, dt)], ident,
                        )
                    mn = psmall.tile([128, 1], f32, tag="mn")
                    nc.vector.tensor_reduce(
                        out=mn, in_=ep, axis=mybir.AxisListType.X,
                        op=mybir.AluOpType.min,
                    )
                    a_ = patt.tile([128, C], f16, tag=f"att{ct}")
                    ss = psmall.tile([128, 1], f32, tag="ss")
                    nc.scalar.activation(
                        out=a_, in_=ep,
                        func=mybir.ActivationFunctionType.Exp,
                        bias=mn, scale=-1.0, accum_out=ss,
                    )
                    rg = psmall.tile([128, 1], f32, tag="rg")
                    nc.vector.reciprocal(out=rg, in_=ss)
                    nc.vector.tensor_mul(out=rg, in0=rg, in1=gam)
                    nc.vector.tensor_scalar_mul(a_, a_, rg)
                    att[ct] = a_

                # ---- A: transpose att -> attT (fp16, per-dt tiles) ----
                attT = []
                for dt in range(CT):
                    tp = ptp.tile([128, C], f16, tag="tp")
                    for ct in range(CT):
                        nc.tensor.transpose(
                            tp[:, ct * 128:(ct + 1) * 128],
                            att[ct][:, dt * 128:(dt + 1) * 128],
                            ident16,
                        )
                    aT = patt.tile([128, C], f16, tag=f"attT{dt}")
                    nc.vector.tensor_copy(out=aT, in_=tp)
                    attT.append(aT)

                # ---- O: out = attT.T @ chunks + x, per 512-wide n-chunk ----
                # last n-iter x for ct3 comes from a re-streamed tile so
                # v3's second half releases one iteration early
                xs3 = pstream.tile([128, 512], f32, tag="xs3")
                nc.sync.dma_start(
                    out=xs3, in_=x_d[b, 384:512, (NCH - 1) * 512:],
                )

                def xsrc(dt, n):
                    if n == NCH - 1 and dt == 3:
                        return xs3
                    return xcol(dt, n)

                for n in range(NCH):
                    nsl = slice(n * 512, (n + 1) * 512)
                    chunks = []
                    for dt in range(CT):
                        ch = pchunk.tile([128, 512], f16, tag=f"ch{dt}")
                        nc.gpsimd.tensor_copy(out=ch, in_=xsrc(dt, n))
                        chunks.append(ch)
                    for ct in range(CT):
                        op = pop.tile([128, 512], f32, tag="op")
                        for dt in range(CT):
                            nc.tensor.matmul(
                                op,
                                lhsT=attT[dt][:, ct * 128:(ct + 1) * 128],
                                rhs=chunks[dt],
                                start=(dt == 0),
                                stop=(dt == CT - 1),
                            )
                        st = pstage.tile([128, 512], f32, tag="st")
                        nc.vector.tensor_add(out=st, in0=op, in1=xsrc(ct, n))
                        nc.sync.dma_start(
                            out=y_d[b, ct * 128:(ct + 1) * 128, nsl], in_=st,
                        )

    nc.compile()
    return nc


def kernel(x: np.ndarray, gamma: np.ndarray) -> np.ndarray:
    x = np.ascontiguousarray(np.asarray(x, dtype=np.float32))
    gamma = np.ascontiguousarray(np.asarray(gamma, dtype=np.float32))
    B, Cc, H, W = x.shape
    xv = x.reshape(B, Cc, H * W)

    if "nc" not in _CACHE:
        _CACHE["nc"] = _build_nc()
    nc = _CACHE["nc"]

    in_maps = [
        {"x": xv[i * B_PER_CORE:(i + 1) * B_PER_CORE], "gamma": gamma}
        for i in range(N_CORES)
    ]
    res = run_bass_kernel_spmd(nc, in_maps, list(range(N_CORES)))
    y = np.concatenate([res.results[i]["y"] for i in range(N_CORES)], axis=0)
    return y.reshape(B, Cc, H, W).astype(np.float32)


# revision 45
# speedup vs baseline: 42633.0338x; 1.0212x over previous
"""CAM (channel attention module) kernel for Trainium2, 8-core SPMD.

Problem: x (16, 512, 64, 64) f32, gamma (1,) f32.
  v = x.reshape(B, C, N);  E = v @ v.T  (B x 512 x 512)
  att = softmax(rowmax(E) - E)  ==  exp(rowmin(E) - E) / rowsum(...)
  out = gamma * (att @ v) + x

Sharding: data-parallel over batch, 2 batches per core, no collectives.

Per-core per-batch pipeline (matmul operands in fp16 = TF32-class accuracy,
all accumulation and the x-residual in exact f32). Tile-dependency tracking
is whole-tile, so tensors are split into tiles matching their consumers'
granularity (v in halves, att per row-tile, attT per d-tile):
  T: DMA v halves (f32, spread over sync/gpsimd/scalar DMA queues,
     v0/v1/v2 double-buffered cross-batch) -> gpsimd/DVE f32->fp16 copies
     per half -> 128 PE transposes (fp16, 1 cy/row) -> DVE copies to vT.
  E: energy = vT.T @ vT, fp16 operands, f32 PSUM accumulation. Symmetry:
     row-tile ct computes only columns >= ct*128 (N=512/384/256/128, the
     upper triangle); the 6 missing blocks are mirrored from earlier rows
     by PE transpose straight back into the PSUM row.
  S: fused softmax of (rowmin(E) - E) [stable form of softmax(rowmax-E)]:
     DVE rowmin, ACT exp with accumulated rowsum, DVE reciprocal; gamma is
     folded into the per-row scale, so gamma==0 gives att==0 and the output
     is bit-exact x.
  A: 16 PE transposes att -> attT (fp16), per-dt tiles.
  O: out = attT.T @ chunks(fp16 of v) accumulated over d-chunks into f32
     PSUM; DVE epilogue adds the exact-f32 x tile; 256KB stores. The last
     n-iter reads x for ct3 from a re-streamed tile so v3's second half
     releases early for the next batch's load.
"""
import sys

import numpy as np

if "/opt/trn_rl_repo" not in sys.path:
    sys.path.insert(0, "/opt/trn_rl_repo")

import concourse.bass as bass
import concourse.tile as tile
from concourse import bacc, mybir
from concourse.bass_utils import run_bass_kernel_spmd
from concourse.masks import make_identity

N_CORES = 8
B_FULL = 16
B_PER_CORE = B_FULL // N_CORES  # 2
C = 512            # channels
HW = 4096          # H*W
CT = C // 128      # 4 channel tiles
KCH = HW // 128    # 32 contraction chunks for energy
NCH = HW // 512    # 8 output column chunks
HHW = HW // 2      # half of H*W (v half-tile width)

f32 = mybir.dt.float32
f16 = mybir.dt.float16

_CACHE = {}

# engine that converts f32->fp16 for each ct's halves in the T phase
CONV_ENGINES = {0: "vector", 1: "gpsimd", 2: "gpsimd", 3: "gpsimd"}
# DMA queue that loads each ct's halves
LOAD_ENGINES = {0: "sync", 1: "gpsimd", 2: "scalar", 3: "scalar"}


def _build_nc(reps: int = 1):
    nc = bacc.Bacc(None, target_bir_lowering=False)
    x_d = nc.dram_tensor("x", [B_PER_CORE, C, HW], f32, kind="ExternalInput")
    g_d = nc.dram_tensor("gamma", [1], f32, kind="ExternalInput")
    y_d = nc.dram_tensor("y", [B_PER_CORE, C, HW], f32, kind="ExternalOutput")

    with tile.TileContext(nc) as tc:
        with (
            tc.tile_pool(name="pvA", bufs=2) as pvA,        # v0/v1/v2 halves
            tc.tile_pool(name="pvB", bufs=1) as pvB,        # v3 halves
            tc.tile_pool(name="pvt", bufs=1) as pvt,        # vT fp16 32KB
            tc.tile_pool(name="pv16", bufs=2) as pv16,      # fp16 v half-tiles
            tc.tile_pool(name="patt", bufs=1) as patt,      # att/attT fp16
            tc.tile_pool(name="pchunk", bufs=2) as pchunk,  # fp16 rhs chunks
            tc.tile_pool(name="pstage", bufs=4) as pstage,  # out staging
            tc.tile_pool(name="pstream", bufs=1) as pstream,  # x re-stream
            tc.tile_pool(name="psmall", bufs=8) as psmall,  # per-ct scalars
            tc.tile_pool(name="pmir", bufs=1) as pmir,      # mirror blocks
            tc.tile_pool(name="psing", bufs=1) as psing,    # ident, gamma
            tc.tile_pool(name="ptp", bufs=2, space="PSUM") as ptp,
            tc.tile_pool(name="pep", bufs=2, space="PSUM") as pep,
            tc.tile_pool(name="pop", bufs=4, space="PSUM") as pop,
        ):
            ident = psing.tile([128, 128], f32)
            make_identity(nc, ident)
            ident16 = psing.tile([128, 128], f16)
            nc.vector.tensor_copy(out=ident16, in_=ident)
            gam = psing.tile([128, 1], f32)

            def load_gamma():
                g_ap = g_d[:]
                nc.gpsimd.dma_start(
                    out=gam,
                    in_=bass.AP(tensor=g_ap.tensor, offset=g_ap.offset,
                                ap=[[0, 128], [1, 1]]),
                )

            def load_vh(b, ct, h):
                pool = pvB if ct == 3 else pvA
                t_ = pool.tile([128, HHW], f32, tag=f"v{ct}h{h}")
                getattr(nc, LOAD_ENGINES[ct]).dma_start(
                    out=t_,
                    in_=x_d[b, ct * 128:(ct + 1) * 128, h * HHW:(h + 1) * HHW],
                )
                return t_

            def load_group(b):
                tiles = [[None, None] for _ in range(3)]
                for h in (0, 1):
                    for ct in (0, 1, 2):
                        tiles[ct][h] = load_vh(b, ct, h)
                return tiles

            batches = [bb for _ in range(reps) for bb in range(B_PER_CORE)]
            v012 = load_group(batches[0])


            for bi, b in enumerate(batches):
                v = list(v012) + [[load_vh(b, 3, 0), load_vh(b, 3, 1)]]

                def xcol(dt, n):
                    """f32 x slice [128, 512] for (row-tile dt, n-chunk n)."""
                    h, lo = divmod(n * 512, HHW)
                    return v[dt][h][:, lo:lo + 512]

                # ---- T: fp16 convert + PE transposes (1 cy/row).
                # vT is split into 4 k-quarter tiles so the energy chains
                # can start on early quarters while the last quarters'
                # PSUM->SBUF copies are still draining on DVE.
                vTq = [pvt.tile([128, 8, C], f16, tag=f"vTq{q}",
                                name=f"vTq{q}") for q in range(4)]

                def vT(k):
                    return vTq[k // 8][:, k % 8, :]

                for ct in range(CT):
                    eng = getattr(nc, CONV_ENGINES[ct])
                    for h in range(2):
                        v16 = pv16.tile([128, HHW], f16, tag=f"v16_{h}")
                        eng.tensor_copy(out=v16, in_=v[ct][h])
                        for q2 in range(2):
                            tp = ptp.tile([128, 8, 128], f16, tag="tp")
                            for ks in range(8):
                                k = h * 16 + q2 * 8 + ks
                                kl = q2 * 8 + ks
                                nc.tensor.transpose(
                                    tp[:, ks, :],
                                    v16[:, kl * 128:(kl + 1) * 128],
                                    ident16,
                                )
                            nc.vector.tensor_copy(
                                out=vTq[h * 2 + q2][:, :,
                                                    ct * 128:(ct + 1) * 128],
                                in_=tp,
                            )
                if bi == 0:
                    load_gamma()  # emitted late so it never delays v loads
                # prefetch next batch's double-buffered v0/v1/v2
                if bi + 1 < len(batches):
                    v012 = load_group(batches[bi + 1])

                # ---- E + S: energy (fp16 operands, f32 accum) + softmax ----
                att = [None] * CT
                mirror_src = {}  # (ct, dt) -> sbuf [128,128] copy of E block
                for ct in range(CT):
                    off = ct * 128
                    ep = pep.tile([128, C], f32, tag="ep")
                    for k in range(KCH):
                        nc.tensor.matmul(
                            ep[:, off:],
                            lhsT=vT(k)[:, ct * 128:(ct + 1) * 128],
                            rhs=vT(k)[:, off:],
                            start=(k == 0),
                            stop=(k == KCH - 1),
                        )
                    # stash blocks that later rows mirror
                    for (dst, src) in (((1, 0), (0, 1)), ((2, 0), (0, 2)),
                                       ((2, 1), (1, 2)), ((3, 0), (0, 3)),
                                       ((3, 1), (1, 3)), ((3, 2), (2, 3))):
                        if src[0] == ct:
                            sb = pmir.tile([128, 128], f32,
                                           tag=f"mir{dst[0]}{dst[1]}")
                            nc.vector.tensor_copy(
                                out=sb,
                                in_=ep[:, src[1] * 128:(src[1] + 1) * 128],
                            )
                            mirror_src[dst] = sb
                    for dt in range(ct):
                        nc.tensor.transpose(
                            ep[:, dt * 128:(dt + 1) * 128],
                            mirror_src[(ct, dt)], ident,
                        )
                    mn = psmall.tile([128, 1], f32, tag="mn")
                    nc.vector.tensor_reduce(
                        out=mn, in_=ep, axis=mybir.AxisListType.X,
                        op=mybir.AluOpType.min,
                    )
                    a_ = patt.tile([128, C], f16, tag=f"att{ct}")
                    ss = psmall.tile([128, 1], f32, tag="ss")
                    nc.scalar.activation(
                        out=a_, in_=ep,
                        func=mybir.ActivationFunctionType.Exp,
                        bias=mn, scale=-1.0, accum_out=ss,
                    )
                    rg = psmall.tile([128, 1], f32, tag="rg")
                    nc.vector.reciprocal(out=rg, in_=ss)
                    nc.vector.tensor_mul(out=rg, in0=rg, in1=gam)
                    nc.vector.tensor_scalar_mul(a_, a_, rg)
                    att[ct] = a_

                # ---- A: transpose att -> attT (fp16, per-dt tiles).
                # The 4 transpose banks live in the (idle) O-phase PSUM
                # slots; all ct0..2 transposes are emitted first so they
                # execute while softmax(ct3) is still finishing on DVE/ACT.
                atp = [pop.tile([128, CT, 128], f16, tag="op",
                                name=f"atp{dt}") for dt in range(CT)]
                for ct in range(CT - 1):
                    for dt in range(CT):
                        nc.tensor.transpose(
                            atp[dt][:, ct, :],
                            att[ct][:, dt * 128:(dt + 1) * 128],
                            ident16,
                        )
                attT = []
                for dt in range(CT):
                    nc.tensor.transpose(
                        atp[dt][:, CT - 1, :],
                        att[CT - 1][:, dt * 128:(dt + 1) * 128],
                        ident16,
                    )
                    aT = patt.tile([128, CT, 128], f16, tag=f"attT{dt}")
                    nc.vector.tensor_copy(out=aT, in_=atp[dt])
                    attT.append(aT)

                # ---- O: out = attT.T @ chunks + x, per 512-wide n-chunk ----
                # last n-iter x for ct3 comes from a re-streamed tile so
                # v3's second half releases one iteration early
                xs3 = pstream.tile([128, 512], f32, tag="xs3")
                nc.sync.dma_start(
                    out=xs3, in_=x_d[b, 384:512, (NCH - 1) * 512:],
                )

                def xsrc(dt, n):
                    if n == NCH - 1 and dt == 3:
                        return xs3
                    return xcol(dt, n)

                for n in range(NCH):
                    nsl = slice(n * 512, (n + 1) * 512)
                    chunks = []
                    for dt in range(CT):
                        ch = pchunk.tile([128, 512], f16, tag=f"ch{dt}")
                        nc.gpsimd.tensor_copy(out=ch, in_=xsrc(dt, n))
                        chunks.append(ch)
                    for ct in range(CT):
                        op = pop.tile([128, 512], f32, tag="op")
                        for dt in range(CT):
                            nc.tensor.matmul(
                                op,
                                lhsT=attT[dt][:, ct, :],
                                rhs=chunks[dt],
                                start=(dt == 0),
                                stop=(dt == CT - 1),
                            )
                        st = pstage.tile([128, 512], f32, tag="st")
                        nc.vector.tensor_add(out=st, in0=op, in1=xsrc(ct, n))
                        nc.sync.dma_start(
                            out=y_d[b, ct * 128:(ct + 1) * 128, nsl], in_=st,
                        )

    nc.compile()
    return nc


def kernel(x: np.ndarray, gamma: np.ndarray) -> np.ndarray:
    x = np.ascontiguousarray(np.asarray(x, dtype=np.float32))
    gamma = np.ascontiguousarray(np.asarray(gamma, dtype=np.float32))
    B, Cc, H, W = x.shape
    xv = x.reshape(B, Cc, H * W)

    if "nc" not in _CACHE:
        _CACHE["nc"] = _build_nc()
    nc = _CACHE["nc"]

    in_maps = [
        {"x": xv[i * B_PER_CORE:(i + 1) * B_PER_CORE], "gamma": gamma}
        for i in range(N_CORES)
    ]
    res = run_bass_kernel_spmd(nc, in_maps, list(range(N_CORES)))
    y = np.concatenate([res.results[i]["y"] for i in range(N_CORES)], axis=0)
    return y.reshape(B, Cc, H, W).astype(np.float32)


# revision 48
# speedup vs baseline: 42921.5985x; 1.0068x over previous
"""CAM (channel attention module) kernel for Trainium2, 8-core SPMD.

Problem: x (16, 512, 64, 64) f32, gamma (1,) f32.
  v = x.reshape(B, C, N);  E = v @ v.T  (B x 512 x 512)
  att = softmax(rowmax(E) - E)  ==  exp(rowmin(E) - E) / rowsum(...)
  out = gamma * (att @ v) + x

Sharding: data-parallel over batch, 2 batches per core, no collectives.

Per-core per-batch pipeline (matmul operands in fp16 = TF32-class accuracy,
all accumulation and the x-residual in exact f32). Tile-dependency tracking
is whole-tile, so tensors are split into tiles matching their consumers'
granularity (v in quarters, att per row-tile, attT per d-tile):
  T: DMA v quarters (f32, spread over sync/gpsimd/scalar DMA queues,
     v0/v1/v2 double-buffered cross-batch) -> gpsimd/DVE f32->fp16 copies
     per quarter -> 128 PE transposes (fp16, 1 cy/row) -> DVE copies to vT
     (one full fp16 PSUM bank = 8 transposes = one quarter).
  E: energy = vT.T @ vT, fp16 operands, f32 PSUM accumulation. Symmetry:
     row-tile ct computes only columns >= ct*128 (N=512/384/256/128, the
     upper triangle); the 6 missing blocks are mirrored from earlier rows
     by PE transpose straight back into the PSUM row.
  S: fused softmax of (rowmin(E) - E) [stable form of softmax(rowmax-E)]:
     DVE rowmin, ACT exp with accumulated rowsum, DVE reciprocal; gamma is
     folded into the per-row scale, so gamma==0 gives att==0 and the output
     is bit-exact x.
  A: 16 PE transposes att -> attT (fp16), per-dt tiles.
  O: out = attT.T @ chunks(fp16 of v) accumulated over d-chunks into f32
     PSUM; DVE epilogue adds the exact-f32 x tile; 256KB stores. The last
     n-iter reads x for ct3 from a re-streamed tile so v3's last quarter
     releases early for the next batch's load.
"""
import sys

import numpy as np

if "/opt/trn_rl_repo" not in sys.path:
    sys.path.insert(0, "/opt/trn_rl_repo")

import concourse.bass as bass
import concourse.tile as tile
from concourse import bacc, mybir
from concourse.bass_utils import run_bass_kernel_spmd
from concourse.masks import make_identity

N_CORES = 8
B_FULL = 16
B_PER_CORE = B_FULL // N_CORES  # 2
C = 512            # channels
HW = 4096          # H*W
CT = C // 128      # 4 channel tiles
KCH = HW // 128    # 32 contraction chunks for energy
NCH = HW // 512    # 8 output column chunks
QW = HW // 4       # quarter of H*W (v quarter-tile width)

f32 = mybir.dt.float32
f16 = mybir.dt.float16

_CACHE = {}

# engine that converts f32->fp16 for each ct's halves in the T phase
CONV_ENGINES = {0: "vector", 1: "gpsimd", 2: "gpsimd", 3: "gpsimd"}
# DMA queue that loads each ct's halves
LOAD_ENGINES = {0: "sync", 1: "gpsimd", 2: "scalar", 3: "scalar"}


def _build_nc(reps: int = 1):
    nc = bacc.Bacc(None, target_bir_lowering=False)
    x_d = nc.dram_tensor("x", [B_PER_CORE, C, HW], f32, kind="ExternalInput")
    g_d = nc.dram_tensor("gamma", [1], f32, kind="ExternalInput")
    y_d = nc.dram_tensor("y", [B_PER_CORE, C, HW], f32, kind="ExternalOutput")

    with tile.TileContext(nc) as tc:
        with (
            tc.tile_pool(name="pvA", bufs=2) as pvA,        # v0/v1/v2 halves
            tc.tile_pool(name="pvB", bufs=1) as pvB,        # v3 halves
            tc.tile_pool(name="pvt", bufs=1) as pvt,        # vT fp16 32KB
            tc.tile_pool(name="pv16", bufs=2) as pv16,      # fp16 v half-tiles
            tc.tile_pool(name="patt", bufs=1) as patt,      # att/attT fp16
            tc.tile_pool(name="pchunk", bufs=2) as pchunk,  # fp16 rhs chunks
            tc.tile_pool(name="pstage", bufs=4) as pstage,  # out staging
            tc.tile_pool(name="pstream", bufs=1) as pstream,  # x re-stream
            tc.tile_pool(name="psmall", bufs=8) as psmall,  # per-ct scalars
            tc.tile_pool(name="pmir", bufs=1) as pmir,      # mirror blocks
            tc.tile_pool(name="psing", bufs=1) as psing,    # ident, gamma
            tc.tile_pool(name="ptp", bufs=2, space="PSUM") as ptp,
            tc.tile_pool(name="pep", bufs=2, space="PSUM") as pep,
            tc.tile_pool(name="pop", bufs=4, space="PSUM") as pop,
        ):
            ident = psing.tile([128, 128], f32)
            make_identity(nc, ident)
            ident16 = psing.tile([128, 128], f16)
            nc.vector.tensor_copy(out=ident16, in_=ident)
            gam = psing.tile([128, 1], f32)

            def load_gamma():
                g_ap = g_d[:]
                nc.gpsimd.dma_start(
                    out=gam,
                    in_=bass.AP(tensor=g_ap.tensor, offset=g_ap.offset,
                                ap=[[0, 128], [1, 1]]),
                )

            def load_vq(b, ct, q):
                pool = pvB if ct == 3 else pvA
                t_ = pool.tile([128, QW], f32, tag=f"v{ct}q{q}",
                               name=f"v{ct}q{q}")
                getattr(nc, LOAD_ENGINES[ct]).dma_start(
                    out=t_,
                    in_=x_d[b, ct * 128:(ct + 1) * 128, q * QW:(q + 1) * QW],
                )
                return t_

            def load_group(b):
                tiles = [[None] * 4 for _ in range(3)]
                for q in range(4):
                    for ct in (0, 1, 2):
                        tiles[ct][q] = load_vq(b, ct, q)
                return tiles

            batches = [bb for _ in range(reps) for bb in range(B_PER_CORE)]
            v012 = load_group(batches[0])


            for bi, b in enumerate(batches):
                v = list(v012) + [[load_vq(b, 3, q) for q in range(4)]]

                def xcol(dt, n):
                    """f32 x slice [128, 512] for (row-tile dt, n-chunk n)."""
                    q, lo = divmod(n * 512, QW)
                    return v[dt][q][:, lo:lo + 512]

                # ---- T: fp16 convert + PE transposes (1 cy/row).
                # vT is split into 4 k-quarter tiles so the energy chains
                # can start on early quarters while the last quarters'
                # PSUM->SBUF copies are still draining on DVE.
                vTq = [pvt.tile([128, 8, C], f16, tag=f"vTq{q}",
                                name=f"vTq{q}") for q in range(4)]

                def vT(k):
                    return vTq[k // 8][:, k % 8, :]

                for ct in range(CT):
                    eng = getattr(nc, CONV_ENGINES[ct])
                    for q in range(4):
                        v16 = pv16.tile([128, QW], f16, tag=f"v16_{q % 2}")
                        eng.tensor_copy(out=v16, in_=v[ct][q])
                        tp = ptp.tile([128, 8, 128], f16, tag="tp")
                        for ks in range(8):
                            nc.tensor.transpose(
                                tp[:, ks, :],
                                v16[:, ks * 128:(ks + 1) * 128],
                                ident16,
                            )
                        nc.vector.tensor_copy(
                            out=vTq[q][:, :, ct * 128:(ct + 1) * 128],
                            in_=tp,
                        )
                if bi == 0:
                    load_gamma()  # emitted late so it never delays v loads
                # prefetch next batch's double-buffered v0/v1/v2
                if bi + 1 < len(batches):
                    v012 = load_group(batches[bi + 1])

                # ---- E + S: energy (fp16 operands, f32 accum) + softmax ----
                att = [None] * CT
                mirror_src = {}  # (ct, dt) -> sbuf [128,128] copy of E block
                for ct in range(CT):
                    off = ct * 128
                    ep = pep.tile([128, C], f32, tag="ep")
                    for k in range(KCH):
                        nc.tensor.matmul(
                            ep[:, off:],
                            lhsT=vT(k)[:, ct * 128:(ct + 1) * 128],
                            rhs=vT(k)[:, off:],
                            start=(k == 0),
                            stop=(k == KCH - 1),
                        )
                    # stash blocks that later rows mirror
                    for (dst, src) in (((1, 0), (0, 1)), ((2, 0), (0, 2)),
                                       ((2, 1), (1, 2)), ((3, 0), (0, 3)),
                                       ((3, 1), (1, 3)), ((3, 2), (2, 3))):
                        if src[0] == ct:
                            sb = pmir.tile([128, 128], f32,
                                           tag=f"mir{dst[0]}{dst[1]}")
                            nc.vector.tensor_copy(
                                out=sb,
                                in_=ep[:, src[1] * 128:(src[1] + 1) * 128],
                            )
                            mirror_src[dst] = sb
                    for dt in range(ct):
                        nc.tensor.transpose(
                            ep[:, dt * 128:(dt + 1) * 128],
                            mirror_src[(ct, dt)], ident,
                        )
                    mn = psmall.tile([128, 1], f32, tag="mn")
                    nc.vector.tensor_reduce(
                        out=mn, in_=ep, axis=mybir.AxisListType.X,
                        op=mybir.AluOpType.min,
                    )
                    a_ = patt.tile([128, C], f16, tag=f"att{ct}")
                    ss = psmall.tile([128, 1], f32, tag="ss")
                    nc.scalar.activation(
                        out=a_, in_=ep,
                        func=mybir.ActivationFunctionType.Exp,
                        bias=mn, scale=-1.0, accum_out=ss,
                    )
                    rg = psmall.tile([128, 1], f32, tag="rg")
                    nc.vector.reciprocal(out=rg, in_=ss)
                    nc.vector.tensor_mul(out=rg, in0=rg, in1=gam)
                    nc.vector.tensor_scalar_mul(a_, a_, rg)
                    att[ct] = a_

                # ---- A: transpose att -> attT (fp16, per-dt tiles).
                # The 4 transpose banks live in the (idle) O-phase PSUM
                # slots; all ct0..2 transposes are emitted first so they
                # execute while softmax(ct3) is still finishing on DVE/ACT.
                atp = [pop.tile([128, CT, 128], f16, tag="op",
                                name=f"atp{dt}") for dt in range(CT)]
                for ct in range(CT - 1):
                    for dt in range(CT):
                        nc.tensor.transpose(
                            atp[dt][:, ct, :],
                            att[ct][:, dt * 128:(dt + 1) * 128],
                            ident16,
                        )
                attT = []
                for dt in range(CT):
                    nc.tensor.transpose(
                        atp[dt][:, CT - 1, :],
                        att[CT - 1][:, dt * 128:(dt + 1) * 128],
                        ident16,
                    )
                    aT = patt.tile([128, CT, 128], f16, tag=f"attT{dt}")
                    nc.vector.tensor_copy(out=aT, in_=atp[dt])
                    attT.append(aT)

                # ---- O: out = attT.T @ chunks + x, per 512-wide n-chunk ----
                # last n-iter x for ct3 comes from a re-streamed tile so
                # v3's second half releases one iteration early
                xs3 = pstream.tile([128, 512], f32, tag="xs3")
                nc.sync.dma_start(
                    out=xs3, in_=x_d[b, 384:512, (NCH - 1) * 512:],
                )

                def xsrc(dt, n):
                    if n == NCH - 1 and dt == 3:
                        return xs3
                    return xcol(dt, n)

                for n in range(NCH):
                    nsl = slice(n * 512, (n + 1) * 512)
                    chunks = []
                    for dt in range(CT):
                        ch = pchunk.tile([128, 512], f16, tag=f"ch{dt}")
                        nc.gpsimd.tensor_copy(out=ch, in_=xsrc(dt, n))
                        chunks.append(ch)
                    for ct in range(CT):
                        op = pop.tile([128, 512], f32, tag="op")
                        for dt in range(CT):
                            nc.tensor.matmul(
                                op,
                                lhsT=attT[dt][:, ct, :],
                                rhs=chunks[dt],
                                start=(dt == 0),
                                stop=(dt == CT - 1),
                            )
                        st = pstage.tile([128, 512], f32, tag="st")
                        nc.vector.tensor_add(out=st, in0=op, in1=xsrc(ct, n))
                        nc.sync.dma_start(
                            out=y_d[b, ct * 128:(ct + 1) * 128, nsl], in_=st,
                        )

    nc.compile()
    return nc


def kernel(x: np.ndarray, gamma: np.ndarray) -> np.ndarray:
    x = np.ascontiguousarray(np.asarray(x, dtype=np.float32))
    gamma = np.ascontiguousarray(np.asarray(gamma, dtype=np.float32))
    B, Cc, H, W = x.shape
    xv = x.reshape(B, Cc, H * W)

    if "nc" not in _CACHE:
        _CACHE["nc"] = _build_nc()
    nc = _CACHE["nc"]

    in_maps = [
        {"x": xv[i * B_PER_CORE:(i + 1) * B_PER_CORE], "gamma": gamma}
        for i in range(N_CORES)
    ]
    res = run_bass_kernel_spmd(nc, in_maps, list(range(N_CORES)))
    y = np.concatenate([res.results[i]["y"] for i in range(N_CORES)], axis=0)
    return y.reshape(B, Cc, H, W).astype(np.float32)
